# revision 18
# baseline (speedup 1.0000x reference)
"""Trainium2 Bass kernel for nn_ActorGCN (GCNConv -> BatchNorm -> Linear ->
ReLU -> softmax -> mask), sharded over 8 NeuronCores.

v2' strategy (dst-partitioned graph parallel, multi-queue gather):
  * Nodes packed into 216 windows of 64 dst slots per core (13824 slots >=
    12500 real nodes/core), windows grouped 24-per-group (9 groups).
  * Edges (incl self-loops) dst-segmented per (group, src-block, window);
    each 128-edge chunk-col gathers its 128 source rows x[src] (fp16, 256B)
    with gpsimd dma_gather.  Gathers are spread round-robin over the 4 SWDGE
    queues so all 8 Q7 descriptor-gen cores work concurrently (the v1 kernel
    used queue 0 only, serializing on 2 of the 8 cores).
  * Host-built weighted one-hots [128 edges x 64 dsts] (fp16, w_e =
    isd[src]*isd[dst] folded in) are DMA'd and multiplied against the
    gathered rows on the TensorEngine, accumulating into per-window PSUM
    regions (8 windows per 2KB bank).  Every PSUM accumulation group opens
    and closes within one gather call; cross-call accumulation runs on the
    Vector engine into an SBUF accumulator gacc[100, 1536] per group.
  * Per group gacc is pushed through W (fp16) into agg[128, 13824] (fp16).
  * BatchNorm batch stats AllReduced across the 8 cores (2x128 floats); BN +
    Linear folded into W2 = scale*Wlin and C = shift@Wlin + blin;
    relu+softmax(2)+mask run as a batched tail.
  * xpad is spread over 4 equal 32768-row blocks (row = (n%4)*32768 + n//4)
    so int16 gather indices reach every block and the 4 per-group gather
    calls are equal-sized.  Slot padding uses idx=0 with a zero one-hot
    column (gathers a real row, contributes nothing).
"""
import sys

sys.path.insert(0, "/opt/trn_rl_repo")

import numpy as np

N = 100000
E = 3200000
F_IN = 100
H = 128
OUT = 2
EPS = 1e-5
NCORES = 8
P = 128              # edge slots per chunk-col
W = 64               # dst slots per window
NWIN = 216           # windows per core (216*64 = 13824 >= 12500)
WPG = 12             # windows per group
NGRP = NWIN // WPG   # 18 groups
GW = WPG * W         # 768 agg cols per group
NB = (GW + 511) // 512   # psum bank-tiles per group
# SWDGE ring holds 1024 descs/engine/queue; a call is callcols*8 descs, so
# keep callcols <= ~60 so two calls fit in the ring (no sequencer stall).
NPAIR = NWIN // 2    # 108 pairs (tail layout)
PC = 128
NBLK = 4
BROWS = 32768        # rows per xpad block
XROWS = NBLK * BROWS
NWG = NWIN * NCORES  # 1728 global windows
DSLOT = NWIN * W     # 13824 dst slots per core
NQ = 4               # SWDGE queues

_cache = {}


def _prep(edge_index):
    """Pure-structure host prep: window assignment, edge segmentation,
    idx + one-hot streams.  Does NOT touch node features."""
    src_e = edge_index[0].astype(np.int64)
    dst_e = edge_index[1].astype(np.int64)

    deg = np.bincount(dst_e, minlength=N).astype(np.int64) + 1
    isd = 1.0 / np.sqrt(deg.astype(np.float64))

    loops = np.arange(N, dtype=np.int64)
    src_all = np.concatenate([src_e, loops])
    dst_all = np.concatenate([dst_e, loops])
    w_all = (isd[src_all] * isd[dst_all]).astype(np.float32)

    # ---- node -> (core, window, row): snake deal by degree ----
    order = np.argsort(-deg, kind="stable")
    i = np.arange(N)
    cyc = i % NWG
    rnd = i // NWG
    wr = np.where(rnd % 2 == 0, cyc, NWG - 1 - cyc)
    core_of = np.empty(N, np.int64)
    win_of = np.empty(N, np.int64)
    row_of = np.empty(N, np.int64)
    core_of[order] = wr % NCORES
    win_of[order] = wr // NCORES
    row_of[order] = rnd
    assert row_of.max() < W

    # tail layout: pair = win//2, poscol = (win%2)*64 + row
    pair_of = win_of // 2
    pos_of = (win_of % 2) * W + row_of

    # ---- per-edge attributes ----
    ec = core_of[dst_all]
    ew = win_of[dst_all]
    eg = ew // WPG
    ewl = ew % WPG                         # window within group
    eb = src_all % NBLK
    eidx = (src_all // NBLK).astype(np.int16)
    erow = row_of[dst_all]                 # dst row in window [0,64)

    # ---- segment counts and static column layout ----
    seg = ((ec * NGRP + eg) * NBLK + eb) * WPG + ewl
    cnt = np.bincount(seg, minlength=NCORES * NGRP * NBLK * WPG)
    cnt = cnt.reshape(NCORES, NGRP, NBLK, WPG)
    segcols = -(-cnt.max(axis=0) // P)          # [NGRP, NBLK, WPG]
    callcols = segcols.sum(axis=2)               # [NGRP, NBLK]
    TOTCOLS = int(callcols.sum())

    colbase = np.zeros((NGRP, NBLK, WPG), np.int64)
    callbase = np.zeros((NGRP, NBLK), np.int64)
    icallbase = np.zeros((NGRP, NBLK), np.int64)
    col = 0
    icol = 0
    for g in range(NGRP):
        for b in range(NBLK):
            callbase[g, b] = col
            icallbase[g, b] = icol
            for wl in range(WPG):
                colbase[g, b, wl] = col
                col += segcols[g, b, wl]
            icol += int(callcols[g, b]) * P // 16
    ICOLS = icol

    # ---- per-core streams ----
    idx16 = np.zeros((NCORES, 128, ICOLS), np.int16)
    oh = np.zeros((NCORES, 128, TOTCOLS, W), np.float16)

    eorder = np.argsort(seg, kind="stable")
    s_seg = seg[eorder]
    s_idx = eidx[eorder]
    s_row = erow[eorder]
    s_w = w_all[eorder]
    s_c = ec[eorder]
    s_g = eg[eorder]
    s_b = eb[eorder]
    s_wl = ewl[eorder]

    nseg = NCORES * NGRP * NBLK * WPG
    seg_start = np.zeros(nseg + 1, np.int64)
    np.cumsum(np.bincount(s_seg, minlength=nseg), out=seg_start[1:])
    j_in_seg = np.arange(len(s_seg)) - seg_start[s_seg]

    e_col = colbase[s_g, s_b, s_wl] + j_in_seg // P
    e_prt = j_in_seg % P
    e_lin = (e_col - callbase[s_g, s_b]) * P + e_prt
    e_ic = icallbase[s_g, s_b] + e_lin // 16
    e_ip = e_lin % 16

    idx16[s_c, e_ip, e_ic] = s_idx
    oh[s_c, e_prt, e_col, s_row] = s_w.astype(np.float16)
    for r in range(1, 8):
        idx16[:, 16 * r:16 * (r + 1), :] = idx16[:, 0:16, :]

    return dict(
        core_of=core_of, pair_of=pair_of, row_of=pos_of,
        segcols=segcols, callcols=callcols, colbase=colbase,
        callbase=callbase, icallbase=icallbase,
        TOTCOLS=TOTCOLS, ICOLS=ICOLS,
        idx16=idx16, oh=oh,
    )


def _build(meta):
    """Build the SPMD Bass/Tile program (identical for all 8 cores)."""
    from concourse import bass, bacc, mybir, tile

    segcols = meta["segcols"]
    callcols = meta["callcols"]
    colbase = meta["colbase"]
    callbase = meta["callbase"]
    icallbase = meta["icallbase"]
    TOTCOLS = meta["TOTCOLS"]
    ICOLS = meta["ICOLS"]
    f16 = mybir.dt.float16
    f32 = mybir.dt.float32

    nc = bacc.Bacc("TRN2", target_bir_lowering=False, debug=False,
                   num_swdge_queues=NQ)
    xpad = nc.dram_tensor("xpad", [XROWS, 128], f16, kind="ExternalInput")
    idxd = nc.dram_tensor("idx", [128, ICOLS], mybir.dt.int16,
                          kind="ExternalInput")
    ohd = nc.dram_tensor("oh", [128, TOTCOLS * W], f16, kind="ExternalInput")
    wmatd = nc.dram_tensor("wmat", [F_IN, H], f16, kind="ExternalInput")
    gamd = nc.dram_tensor("gam", [H, 1], f32, kind="ExternalInput")
    betd = nc.dram_tensor("bet", [H, 1], f32, kind="ExternalInput")
    wlind = nc.dram_tensor("wlin", [H, OUT], f32, kind="ExternalInput")
    blind = nc.dram_tensor("blin", [1, OUT], f32, kind="ExternalInput")
    maskd = nc.dram_tensor("mask", [128, NPAIR * 2], f32, kind="ExternalInput")
    outd = nc.dram_tensor("out", [128, NPAIR * 2], f32, kind="ExternalOutput")

    def bcast_mid(ap, n):
        a = ap.ap
        assert len(a) == 2
        return bass.AP(ap.tensor, ap.offset, [a[0], [0, n], a[1]])

    def bcast_last(ap, n):
        a = ap.ap
        assert len(a) == 2
        return bass.AP(ap.tensor, ap.offset, [a[0], a[1], [0, n]])

    with tile.TileContext(nc) as tc:
        with (
            tc.tile_pool(name="const", bufs=1) as cp,
            tc.tile_pool(name="agg", bufs=1) as aggp,
            tc.tile_pool(name="idxp", bufs=12) as ip,
            tc.tile_pool(name="xp", bufs=7) as xp,
            tc.tile_pool(name="ohp", bufs=4) as ohp,
            tc.tile_pool(name="gaccp", bufs=2) as gp,
            tc.tile_pool(name="small", bufs=1) as sp,
            tc.tile_pool(name="pairp", bufs=2, space="PSUM") as prp,
            tc.tile_pool(name="stp", bufs=1, space="PSUM") as stp,
            tc.tile_pool(name="logp", bufs=1, space="PSUM") as lgp,
            tc.tile_pool(name="dram", bufs=1, space="DRAM") as dr,
        ):
            wmat_t = cp.tile([F_IN, H], f16)
            nc.sync.dma_start(out=wmat_t[:], in_=wmatd[:])
            gam_t = cp.tile([H, 1], f32)
            nc.sync.dma_start(out=gam_t[:], in_=gamd[:])
            bet_t = cp.tile([H, 1], f32)
            nc.sync.dma_start(out=bet_t[:], in_=betd[:])
            wlin_t = cp.tile([H, OUT], f32)
            nc.sync.dma_start(out=wlin_t[:], in_=wlind[:])
            blin_t = cp.tile([1, OUT], f32)
            nc.sync.dma_start(out=blin_t[:], in_=blind[:])
            mask_t = cp.tile([128, NPAIR * 2], f32)
            nc.sync.dma_start(out=mask_t[:], in_=maskd[:])

            agg = aggp.tile([128, NWIN * W], f16)
            Lt = aggp.tile([128, NPAIR * 2], f32)

            # ---------------- main loop: gather + one-hot matmuls ----------
            for g in range(NGRP):
                gacc = gp.tile([128, WPG * W], f32, tag="gacc",
                               name=f"gacc{g}")
                nc.vector.memset(gacc[:], 0.0)
                for b in range(NBLK):
                    cc = int(callcols[g, b])
                    if cc == 0:
                        continue
                    cb = int(callbase[g, b])
                    icb = int(icallbase[g, b])
                    idx_t = ip.tile([128, cc * P // 16], mybir.dt.int16,
                                    tag="idx", name=f"idx_{g}_{b}")
                    nc.sync.dma_start(out=idx_t[:],
                                      in_=idxd[:, icb:icb + cc * P // 16])
                    xall = xp.tile([128, cc, 128], f16, tag="xall",
                                   name=f"xall_{g}_{b}")
                    nc.gpsimd.dma_gather(
                        out_ap=xall[:],
                        in_ap=xpad[b * BROWS:(b + 1) * BROWS, :],
                        idxs_ap=idx_t[:],
                        num_idxs=cc * P,
                        num_idxs_reg=cc * P,
                        elem_size=128,
                        single_packet=False,
                        # Tile assigns SWDGE sem lanes round-robin by emission
                        # order (mod 8); queue must track emission order mod 4
                        # so each lane sees one queue only.  Blocks are
                        # equal-sized, so queue=b stays load-balanced.
                        queue_num=(g * NBLK + b) % NQ,
                    )
                    oh_t = ohp.tile([128, cc, W], f16, tag="oh",
                                    name=f"oh_{g}_{b}")
                    nc.sync.dma_start(
                        out=oh_t[:],
                        in_=ohd[:, cb * W:(cb + cc) * W].rearrange(
                            "p (c w) -> p c w", w=W))
                    psb = [prp.tile([128, 512], f32, tag=f"psb{k}",
                                    name=f"psb{k}_{g}_{b}") for k in range(NB)]
                    for wl in range(WPG):
                        ncols = int(segcols[g, b, wl])
                        if ncols == 0:
                            continue
                        c0 = int(colbase[g, b, wl]) - cb
                        for k in range(ncols):
                            nc.tensor.matmul(
                                psb[wl // 8][0:F_IN,
                                             (wl % 8) * W:(wl % 8 + 1) * W],
                                xall[:, c0 + k, 0:F_IN],
                                oh_t[:, c0 + k, :],
                                start=(k == 0), stop=(k == ncols - 1))
                    # accumulate this call's psum banks into gacc (SBUF)
                    for k in range(NB):
                        lo = k * 512
                        hi = min((k + 1) * 512, GW)
                        has = any(segcols[g, b, wl] > 0
                                  for wl in range(k * 8, min(k * 8 + 8, WPG)))
                        if not has:
                            continue
                        nc.vector.tensor_tensor(
                            out=gacc[0:F_IN, lo:hi],
                            in0=psb[k][0:F_IN, 0:hi - lo],
                            in1=gacc[0:F_IN, lo:hi],
                            op=mybir.AluOpType.add)

                # ---- stage 2: through W into agg ----
                gsb = gp.tile([F_IN, GW], f16, tag="gsb", name=f"gsb{g}")
                nc.scalar.copy(out=gsb[:], in_=gacc[0:F_IN, :])
                for k in range(NB):
                    lo = k * 512
                    hi = min((k + 1) * 512, GW)
                    st = stp.tile([H, 512], f32, tag="st", name=f"st_{g}_{k}")
                    nc.tensor.matmul(st[0:H, 0:hi - lo], wmat_t[:],
                                     gsb[:, lo:hi],
                                     start=True, stop=True)
                    nc.scalar.copy(
                        out=agg[:, g * GW + lo:g * GW + hi],
                        in_=st[0:H, 0:hi - lo])

            # ---------------- batch-norm statistics + AllReduce ------------
            ssum = sp.tile([H, 1], f32)
            nc.vector.tensor_reduce(out=ssum[:], in_=agg[:],
                                    axis=mybir.AxisListType.X,
                                    op=mybir.AluOpType.add)
            qsum = sp.tile([H, 1], f32)
            sqch = 864
            nsq = NWIN * W // sqch
            sq_scratch = sp.tile([H, sqch], f32)
            qpart = sp.tile([H, 1], f32)
            for i in range(nsq):
                nc.scalar.activation(
                    out=sq_scratch[:], in_=agg[:, i * sqch:(i + 1) * sqch],
                    func=mybir.ActivationFunctionType.Square,
                    accum_out=qpart[:] if i else qsum[:])
                if i:
                    nc.vector.tensor_tensor(out=qsum[:], in0=qsum[:],
                                            in1=qpart[:],
                                            op=mybir.AluOpType.add)

            packed = sp.tile([H, 2], f32)
            nc.vector.tensor_copy(out=packed[:, 0:1], in_=ssum[:])
            nc.vector.tensor_copy(out=packed[:, 1:2], in_=qsum[:])
            ib = dr.tile([H, 2], f32)
            ob = dr.tile([H, 2], f32)
            nc.gpsimd.dma_start(out=ib[:], in_=packed[:])
            nc.gpsimd.collective_compute(
                "AllReduce", mybir.AluOpType.add,
                replica_groups=[list(range(NCORES))],
                ins=[ib.opt()], outs=[ob.opt()])
            res = sp.tile([H, 2], f32)
            nc.sync.dma_start(out=res[:], in_=ob[:])

            mean = sp.tile([H, 1], f32)
            nc.vector.tensor_scalar(out=mean[:], in0=res[:, 0:1],
                                    scalar1=1.0 / N, scalar2=None,
                                    op0=mybir.AluOpType.mult)
            ex2 = sp.tile([H, 1], f32)
            nc.vector.tensor_scalar(out=ex2[:], in0=res[:, 1:2],
                                    scalar1=1.0 / N, scalar2=None,
                                    op0=mybir.AluOpType.mult)
            msq = sp.tile([H, 1], f32)
            nc.vector.tensor_tensor(out=msq[:], in0=mean[:], in1=mean[:],
                                    op=mybir.AluOpType.mult)
            var = sp.tile([H, 1], f32)
            nc.vector.tensor_tensor(out=var[:], in0=ex2[:], in1=msq[:],
                                    op=mybir.AluOpType.subtract)
            vare = sp.tile([H, 1], f32)
            nc.vector.tensor_scalar(out=vare[:], in0=var[:], scalar1=EPS,
                                    scalar2=None, op0=mybir.AluOpType.add)
            std = sp.tile([H, 1], f32)
            nc.scalar.activation(out=std[:], in_=vare[:],
                                 func=mybir.ActivationFunctionType.Sqrt)
            inv = sp.tile([H, 1], f32)
            nc.vector.reciprocal(inv[:], std[:])
            scale = sp.tile([H, 1], f32)
            nc.vector.tensor_tensor(out=scale[:], in0=gam_t[:], in1=inv[:],
                                    op=mybir.AluOpType.mult)
            mscale = sp.tile([H, 1], f32)
            nc.vector.tensor_tensor(out=mscale[:], in0=mean[:], in1=scale[:],
                                    op=mybir.AluOpType.mult)
            shift = sp.tile([H, 1], f32)
            nc.vector.tensor_tensor(out=shift[:], in0=bet_t[:], in1=mscale[:],
                                    op=mybir.AluOpType.subtract)
            w2 = sp.tile([H, OUT], f32)
            nc.vector.tensor_scalar(out=w2[:], in0=wlin_t[:], scalar1=scale[:],
                                    scalar2=None, op0=mybir.AluOpType.mult)
            w2h = sp.tile([H, OUT], f16)
            nc.vector.tensor_copy(out=w2h[:], in_=w2[:])
            psc = stp.tile([1, OUT], f32, tag="st", name="psc")
            nc.tensor.matmul(psc[:], shift[:], wlin_t[:], start=True, stop=True)
            cvec = sp.tile([1, OUT], f32)
            nc.vector.tensor_tensor(out=cvec[:], in0=psc[:], in1=blin_t[:],
                                    op=mybir.AluOpType.add)
            ones_t = sp.tile([1, 128], f32)
            nc.vector.memset(ones_t[:], 1.0)
            pscb = stp.tile([128, OUT], f32, tag="st", name="pscb")
            nc.tensor.matmul(pscb[:], ones_t[:], cvec[:], start=True, stop=True)
            cb2 = sp.tile([128, OUT], f32)
            nc.vector.tensor_copy(out=cb2[:], in_=pscb[:])

            # ---------------- logits + batched softmax tail ----------------
            lg = lgp.tile([128, NPAIR * 2], f32, tag="lg")
            for pg in range(NPAIR):
                nc.tensor.matmul(lg[:, pg * 2:(pg + 1) * 2],
                                 agg[:, pg * PC:(pg + 1) * PC], w2h[:],
                                 start=True, stop=True)
            nc.scalar.copy(out=Lt[:], in_=lg[:])

            lc2 = aggp.tile([128, NPAIR * 2], f32)
            nc.vector.tensor_tensor(
                out=lc2[:].rearrange("p (k o) -> p k o", o=2),
                in0=Lt[:].rearrange("p (k o) -> p k o", o=2),
                in1=bcast_mid(cb2[:], NPAIR),
                op=mybir.AluOpType.add)
            ee = aggp.tile([128, NPAIR * 2], f32)
            nc.scalar.activation(out=ee[:], in_=lc2[:],
                                 func=mybir.ActivationFunctionType.Exp)
            # exp(relu(x)) = max(exp(x), 1)
            nc.vector.tensor_scalar(out=ee[:], in0=ee[:], scalar1=1.0,
                                    scalar2=None, op0=mybir.AluOpType.max)
            ssm = aggp.tile([128, NPAIR], f32)
            nc.vector.tensor_reduce(
                out=ssm[:], in_=ee[:].rearrange("p (k o) -> p k o", o=2),
                axis=mybir.AxisListType.X, op=mybir.AluOpType.add)
            rin = aggp.tile([128, NPAIR], f32)
            nc.vector.reciprocal(rin[:], ssm[:])
            rm = aggp.tile([128, NPAIR * 2], f32)
            nc.vector.tensor_tensor(
                out=rm[:].rearrange("p (k o) -> p k o", o=2),
                in0=bcast_last(rin[:], 2),
                in1=mask_t[:].rearrange("p (k o) -> p k o", o=2),
                op=mybir.AluOpType.mult)
            pf = aggp.tile([128, NPAIR * 2], f32)
            nc.vector.tensor_tensor(out=pf[:], in0=ee[:], in1=rm[:],
                                    op=mybir.AluOpType.mult)
            nc.sync.dma_start(out=outd[:], in_=pf[:])

    nc.finalize()
    return nc


def build_in_maps(inputs, meta):
    state = np.asarray(inputs["state"], dtype=np.float32)
    Wm = np.asarray(inputs["W"], dtype=np.float32)
    gamma = np.asarray(inputs["gamma"], dtype=np.float32)
    beta = np.asarray(inputs["beta"], dtype=np.float32)
    Wlin = np.asarray(inputs["Wlin"], dtype=np.float32)
    blin = np.asarray(inputs["blin"], dtype=np.float32)
    mask = np.asarray(inputs["mask"])

    x = state.reshape(N, F_IN)
    # xpad layout: node n -> row (n%4)*BROWS + n//4
    xpad = np.zeros((XROWS, 128), dtype=np.float16)
    rows = (np.arange(N) % NBLK) * BROWS + np.arange(N) // NBLK
    xpad[rows, :F_IN] = x.astype(np.float16)

    core_of, pair_of, row_of = (meta["core_of"], meta["pair_of"],
                                meta["row_of"])
    maskc = np.zeros((NCORES, 128, NPAIR * 2), dtype=np.float32)
    mf = mask.astype(np.float32)
    maskc[core_of, row_of, pair_of * 2] = mf
    maskc[core_of, row_of, pair_of * 2 + 1] = mf

    in_maps = []
    for c in range(NCORES):
        in_maps.append(dict(
            xpad=xpad,
            idx=meta["idx16"][c],
            oh=meta["oh"][c].reshape(128, -1),
            mask=maskc[c],
            wmat=Wm.astype(np.float16),
            gam=gamma.reshape(H, 1),
            bet=beta.reshape(H, 1),
            wlin=Wlin,
            blin=blin.reshape(1, OUT),
        ))
    return in_maps


def unpack_out(meta, outs):
    core_of, pair_of, row_of = (meta["core_of"], meta["pair_of"],
                                meta["row_of"])
    nds = np.arange(N)
    actor = np.zeros((N, OUT), dtype=np.float32)
    for c in range(NCORES):
        o = outs[c]                                 # [128, NPAIR*2]
        sel = core_of == c
        actor[nds[sel], 0] = o[row_of[sel], pair_of[sel] * 2]
        actor[nds[sel], 1] = o[row_of[sel], pair_of[sel] * 2 + 1]
    return actor


def kernel(**inputs):
    edge_index = np.asarray(inputs["edge_index"])
    meta = _prep(edge_index)

    key = tuple(meta["segcols"].reshape(-1).tolist())
    if key not in _cache:
        _cache[key] = _build(meta)
    nc = _cache[key]

    in_maps = build_in_maps(inputs, meta)
    global _last_in_maps
    _last_in_maps = in_maps
    import os
    from concourse.bass_utils import run_bass_kernel_spmd
    if os.environ.get("KERNEL_TRACE"):
        import tempfile
        tcores = None
        if os.environ.get("KERNEL_TRACE_CORES"):
            tcores = [int(c) for c in
                      os.environ["KERNEL_TRACE_CORES"].split(",")]
        r = run_bass_kernel_spmd(nc, in_maps, list(range(NCORES)), trace=True,
                                 trace_cores=tcores,
                                 tmpdir=tempfile.mkdtemp(prefix="ktrace_"))
        print(f"HW exec time: {r.exec_time_ns} ns")
    else:
        r = run_bass_kernel_spmd(nc, in_maps, list(range(NCORES)), trace=False)

    return unpack_out(meta, [r.results[c]["out"] for c in range(NCORES)])


# revision 20
# speedup vs baseline: 1.0771x; 1.0771x over previous
"""Trainium2 Bass kernel for nn_ActorGCN (GCNConv -> BatchNorm -> Linear ->
ReLU -> softmax -> mask), sharded over 8 NeuronCores.

v2' strategy (dst-partitioned graph parallel, multi-queue gather):
  * Nodes packed into 216 windows of 64 dst slots per core (13824 slots >=
    12500 real nodes/core), windows grouped 24-per-group (9 groups).
  * Edges (incl self-loops) dst-segmented per (group, src-block, window);
    each 128-edge chunk-col gathers its 128 source rows x[src] (fp16, 256B)
    with gpsimd dma_gather.  Gathers are spread round-robin over the 4 SWDGE
    queues so all 8 Q7 descriptor-gen cores work concurrently (the v1 kernel
    used queue 0 only, serializing on 2 of the 8 cores).
  * Host-built weighted one-hots [128 edges x 64 dsts] (fp16, w_e =
    isd[src]*isd[dst] folded in) are DMA'd and multiplied against the
    gathered rows on the TensorEngine, accumulating into per-window PSUM
    regions (8 windows per 2KB bank).  Every PSUM accumulation group opens
    and closes within one gather call; cross-call accumulation runs on the
    Vector engine into an SBUF accumulator gacc[100, 1536] per group.
  * Per group gacc is pushed through W (fp16) into agg[128, 13824] (fp16).
  * BatchNorm batch stats AllReduced across the 8 cores (2x128 floats); BN +
    Linear folded into W2 = scale*Wlin and C = shift@Wlin + blin;
    relu+softmax(2)+mask run as a batched tail.
  * xpad is spread over 4 equal 32768-row blocks (row = (n%4)*32768 + n//4)
    so int16 gather indices reach every block and the 4 per-group gather
    calls are equal-sized.  Slot padding uses idx=0 with a zero one-hot
    column (gathers a real row, contributes nothing).
"""
import sys

sys.path.insert(0, "/opt/trn_rl_repo")

import numpy as np

N = 100000
E = 3200000
F_IN = 100
H = 128
OUT = 2
EPS = 1e-5
NCORES = 8
P = 128              # edge slots per chunk-col
W = 64               # dst slots per window
NWIN = 216           # windows per core (216*64 = 13824 >= 12500)
WPG = 12             # windows per group
NGRP = NWIN // WPG   # 18 groups
GW = WPG * W         # 768 agg cols per group
NB = (GW + 511) // 512   # psum bank-tiles per group
# SWDGE ring holds 1024 descs/engine/queue; a call is callcols*8 descs, so
# keep callcols <= ~60 so two calls fit in the ring (no sequencer stall).
NPAIR = NWIN // 2    # 108 pairs (tail layout)
PC = 128
NBLK = 4
BROWS = 32768        # rows per xpad block
XROWS = NBLK * BROWS
NWG = NWIN * NCORES  # 1728 global windows
DSLOT = NWIN * W     # 13824 dst slots per core
NQ = 4               # SWDGE queues

_cache = {}


def _prep(edge_index):
    """Pure-structure host prep: window assignment, edge segmentation,
    idx + one-hot streams.  Does NOT touch node features."""
    src_e = edge_index[0].astype(np.int64)
    dst_e = edge_index[1].astype(np.int64)

    deg = np.bincount(dst_e, minlength=N).astype(np.int64) + 1
    isd = 1.0 / np.sqrt(deg.astype(np.float64))

    loops = np.arange(N, dtype=np.int64)
    src_all = np.concatenate([src_e, loops])
    dst_all = np.concatenate([dst_e, loops])
    w_all = (isd[src_all] * isd[dst_all]).astype(np.float32)

    # ---- node -> (core, window, row): snake deal by degree ----
    order = np.argsort(-deg, kind="stable")
    i = np.arange(N)
    cyc = i % NWG
    rnd = i // NWG
    wr = np.where(rnd % 2 == 0, cyc, NWG - 1 - cyc)
    core_of = np.empty(N, np.int64)
    win_of = np.empty(N, np.int64)
    row_of = np.empty(N, np.int64)
    core_of[order] = wr % NCORES
    win_of[order] = wr // NCORES
    row_of[order] = rnd
    assert row_of.max() < W

    # tail layout: pair = win//2, poscol = (win%2)*64 + row
    pair_of = win_of // 2
    pos_of = (win_of % 2) * W + row_of

    # ---- per-edge attributes ----
    ec = core_of[dst_all]
    ew = win_of[dst_all]
    eg = ew // WPG
    ewl = ew % WPG                         # window within group
    eb = src_all % NBLK
    eidx = (src_all // NBLK).astype(np.int16)
    erow = row_of[dst_all]                 # dst row in window [0,64)

    # ---- segment counts and static column layout ----
    seg = ((ec * NGRP + eg) * NBLK + eb) * WPG + ewl
    cnt = np.bincount(seg, minlength=NCORES * NGRP * NBLK * WPG)
    cnt = cnt.reshape(NCORES, NGRP, NBLK, WPG)
    segcols = -(-cnt.max(axis=0) // P)          # [NGRP, NBLK, WPG]
    callcols = segcols.sum(axis=2)               # [NGRP, NBLK]
    TOTCOLS = int(callcols.sum())

    colbase = np.zeros((NGRP, NBLK, WPG), np.int64)
    callbase = np.zeros((NGRP, NBLK), np.int64)
    icallbase = np.zeros((NGRP, NBLK), np.int64)
    col = 0
    icol = 0
    for g in range(NGRP):
        for b in range(NBLK):
            callbase[g, b] = col
            icallbase[g, b] = icol
            for wl in range(WPG):
                colbase[g, b, wl] = col
                col += segcols[g, b, wl]
            icol += int(callcols[g, b]) * P // 16
    ICOLS = icol

    # ---- per-core streams ----
    idx16 = np.zeros((NCORES, 128, ICOLS), np.int16)
    oh = np.zeros((NCORES, 128, TOTCOLS, W), np.float16)

    eorder = np.argsort(seg, kind="stable")
    s_seg = seg[eorder]
    s_idx = eidx[eorder]
    s_row = erow[eorder]
    s_w = w_all[eorder]
    s_c = ec[eorder]
    s_g = eg[eorder]
    s_b = eb[eorder]
    s_wl = ewl[eorder]

    nseg = NCORES * NGRP * NBLK * WPG
    seg_start = np.zeros(nseg + 1, np.int64)
    np.cumsum(np.bincount(s_seg, minlength=nseg), out=seg_start[1:])
    j_in_seg = np.arange(len(s_seg)) - seg_start[s_seg]

    e_col = colbase[s_g, s_b, s_wl] + j_in_seg // P
    e_prt = j_in_seg % P
    e_lin = (e_col - callbase[s_g, s_b]) * P + e_prt
    e_ic = icallbase[s_g, s_b] + e_lin // 16
    e_ip = e_lin % 16

    idx16[s_c, e_ip, e_ic] = s_idx
    oh[s_c, e_prt, e_col, s_row] = s_w.astype(np.float16)
    for r in range(1, 8):
        idx16[:, 16 * r:16 * (r + 1), :] = idx16[:, 0:16, :]

    return dict(
        core_of=core_of, pair_of=pair_of, row_of=pos_of,
        segcols=segcols, callcols=callcols, colbase=colbase,
        callbase=callbase, icallbase=icallbase,
        TOTCOLS=TOTCOLS, ICOLS=ICOLS,
        idx16=idx16, oh=oh,
    )


def _build(meta):
    """Build the SPMD Bass/Tile program (identical for all 8 cores)."""
    from concourse import bass, bacc, mybir, tile

    segcols = meta["segcols"]
    callcols = meta["callcols"]
    colbase = meta["colbase"]
    callbase = meta["callbase"]
    icallbase = meta["icallbase"]
    TOTCOLS = meta["TOTCOLS"]
    ICOLS = meta["ICOLS"]
    f16 = mybir.dt.float16
    f32 = mybir.dt.float32

    nc = bacc.Bacc("TRN2", target_bir_lowering=False, debug=False,
                   num_swdge_queues=NQ)
    xpad = nc.dram_tensor("xpad", [XROWS, 128], f16, kind="ExternalInput")
    idxd = nc.dram_tensor("idx", [128, ICOLS], mybir.dt.int16,
                          kind="ExternalInput")
    ohd = nc.dram_tensor("oh", [128, TOTCOLS * W], f16, kind="ExternalInput")
    wmatd = nc.dram_tensor("wmat", [F_IN, H], f16, kind="ExternalInput")
    gamd = nc.dram_tensor("gam", [H, 1], f32, kind="ExternalInput")
    betd = nc.dram_tensor("bet", [H, 1], f32, kind="ExternalInput")
    wlind = nc.dram_tensor("wlin", [H, OUT], f32, kind="ExternalInput")
    blind = nc.dram_tensor("blin", [1, OUT], f32, kind="ExternalInput")
    maskd = nc.dram_tensor("mask", [128, NPAIR * 2], f32, kind="ExternalInput")
    outd = nc.dram_tensor("out", [128, NPAIR * 2], f32, kind="ExternalOutput")

    def bcast_mid(ap, n):
        a = ap.ap
        assert len(a) == 2
        return bass.AP(ap.tensor, ap.offset, [a[0], [0, n], a[1]])

    def bcast_last(ap, n):
        a = ap.ap
        assert len(a) == 2
        return bass.AP(ap.tensor, ap.offset, [a[0], a[1], [0, n]])

    with tile.TileContext(nc) as tc:
        with (
            tc.tile_pool(name="const", bufs=1) as cp,
            tc.tile_pool(name="agg", bufs=1) as aggp,
            tc.tile_pool(name="idxp", bufs=3) as ip,
            tc.tile_pool(name="xp", bufs=6) as xp,
            tc.tile_pool(name="ohp", bufs=2) as ohp,
            tc.tile_pool(name="gaccp", bufs=2) as gp,
            tc.tile_pool(name="small", bufs=1) as sp,
            tc.tile_pool(name="pairp", bufs=2, space="PSUM") as prp,
            tc.tile_pool(name="stp", bufs=1, space="PSUM") as stp,
            tc.tile_pool(name="logp", bufs=1, space="PSUM") as lgp,
            tc.tile_pool(name="dram", bufs=1, space="DRAM") as dr,
        ):
            wmat_t = cp.tile([F_IN, H], f16)
            nc.sync.dma_start(out=wmat_t[:], in_=wmatd[:])
            gam_t = cp.tile([H, 1], f32)
            nc.sync.dma_start(out=gam_t[:], in_=gamd[:])
            bet_t = cp.tile([H, 1], f32)
            nc.sync.dma_start(out=bet_t[:], in_=betd[:])
            wlin_t = cp.tile([H, OUT], f32)
            nc.sync.dma_start(out=wlin_t[:], in_=wlind[:])
            blin_t = cp.tile([1, OUT], f32)
            nc.sync.dma_start(out=blin_t[:], in_=blind[:])
            mask_t = cp.tile([128, NPAIR * 2], f32)
            nc.sync.dma_start(out=mask_t[:], in_=maskd[:])

            agg = aggp.tile([128, NWIN * W], f16)
            Lt = aggp.tile([128, NPAIR * 2], f32)

            # ---------------- main loop: gather + one-hot matmuls ----------
            for g in range(NGRP):
                gacc = gp.tile([128, WPG * W], f32, tag="gacc",
                               name=f"gacc{g}")
                nc.vector.memset(gacc[:], 0.0)
                gcb = int(callbase[g, 0])          # first col of group
                gcols = int(callcols[g].sum())      # cols in group
                gicb = int(icallbase[g, 0])
                gicols = gcols * P // 16
                idx_t = ip.tile([128, gicols], mybir.dt.int16,
                                tag="idx", name=f"idx_{g}")
                nc.sync.dma_start(out=idx_t[:],
                                  in_=idxd[:, gicb:gicb + gicols])
                oh_t = ohp.tile([128, gcols, W], f16, tag="oh",
                                name=f"oh_{g}")
                nc.sync.dma_start(
                    out=oh_t[:],
                    in_=ohd[:, gcb * W:(gcb + gcols) * W].rearrange(
                        "p (c w) -> p c w", w=W))
                for b in range(NBLK):
                    cc = int(callcols[g, b])
                    if cc == 0:
                        continue
                    cb = int(callbase[g, b])
                    icb = int(icallbase[g, b])
                    xall = xp.tile([128, cc, 128], f16, tag="xall",
                                   name=f"xall_{g}_{b}")
                    nc.gpsimd.dma_gather(
                        out_ap=xall[:],
                        in_ap=xpad[b * BROWS:(b + 1) * BROWS, :],
                        idxs_ap=idx_t[:, icb - gicb:icb - gicb + cc * P // 16],
                        num_idxs=cc * P,
                        num_idxs_reg=cc * P,
                        elem_size=128,
                        single_packet=False,
                        # Tile assigns SWDGE sem lanes round-robin by emission
                        # order (mod 8); queue must track emission order mod 4
                        # so each lane sees one queue only.  Blocks are
                        # equal-sized, so queue=b stays load-balanced.
                        queue_num=(g * NBLK + b) % NQ,
                    )
                    psb = [prp.tile([128, 512], f32, tag=f"psb{k}",
                                    name=f"psb{k}_{g}_{b}") for k in range(NB)]
                    for wl in range(WPG):
                        ncols = int(segcols[g, b, wl])
                        if ncols == 0:
                            continue
                        c0 = int(colbase[g, b, wl]) - cb
                        for k in range(ncols):
                            nc.tensor.matmul(
                                psb[wl // 8][0:F_IN,
                                             (wl % 8) * W:(wl % 8 + 1) * W],
                                xall[:, c0 + k, 0:F_IN],
                                oh_t[:, cb - gcb + c0 + k, :],
                                start=(k == 0), stop=(k == ncols - 1))
                    # accumulate this call's psum banks into gacc (SBUF)
                    for k in range(NB):
                        lo = k * 512
                        hi = min((k + 1) * 512, GW)
                        has = any(segcols[g, b, wl] > 0
                                  for wl in range(k * 8, min(k * 8 + 8, WPG)))
                        if not has:
                            continue
                        nc.vector.tensor_tensor(
                            out=gacc[0:F_IN, lo:hi],
                            in0=psb[k][0:F_IN, 0:hi - lo],
                            in1=gacc[0:F_IN, lo:hi],
                            op=mybir.AluOpType.add)

                # ---- stage 2: through W into agg ----
                gsb = gp.tile([F_IN, GW], f16, tag="gsb", name=f"gsb{g}")
                nc.scalar.copy(out=gsb[:], in_=gacc[0:F_IN, :])
                for k in range(NB):
                    lo = k * 512
                    hi = min((k + 1) * 512, GW)
                    st = stp.tile([H, 512], f32, tag="st", name=f"st_{g}_{k}")
                    nc.tensor.matmul(st[0:H, 0:hi - lo], wmat_t[:],
                                     gsb[:, lo:hi],
                                     start=True, stop=True)
                    nc.scalar.copy(
                        out=agg[:, g * GW + lo:g * GW + hi],
                        in_=st[0:H, 0:hi - lo])

            # ---------------- batch-norm statistics + AllReduce ------------
            ssum = sp.tile([H, 1], f32)
            nc.vector.tensor_reduce(out=ssum[:], in_=agg[:],
                                    axis=mybir.AxisListType.X,
                                    op=mybir.AluOpType.add)
            qsum = sp.tile([H, 1], f32)
            sqch = 864
            nsq = NWIN * W // sqch
            sq_scratch = sp.tile([H, sqch], f32)
            qpart = sp.tile([H, 1], f32)
            for i in range(nsq):
                nc.scalar.activation(
                    out=sq_scratch[:], in_=agg[:, i * sqch:(i + 1) * sqch],
                    func=mybir.ActivationFunctionType.Square,
                    accum_out=qpart[:] if i else qsum[:])
                if i:
                    nc.vector.tensor_tensor(out=qsum[:], in0=qsum[:],
                                            in1=qpart[:],
                                            op=mybir.AluOpType.add)

            packed = sp.tile([H, 2], f32)
            nc.vector.tensor_copy(out=packed[:, 0:1], in_=ssum[:])
            nc.vector.tensor_copy(out=packed[:, 1:2], in_=qsum[:])
            ib = dr.tile([H, 2], f32)
            ob = dr.tile([H, 2], f32)
            nc.gpsimd.dma_start(out=ib[:], in_=packed[:])
            nc.gpsimd.collective_compute(
                "AllReduce", mybir.AluOpType.add,
                replica_groups=[list(range(NCORES))],
                ins=[ib.opt()], outs=[ob.opt()])
            res = sp.tile([H, 2], f32)
            nc.sync.dma_start(out=res[:], in_=ob[:])

            mean = sp.tile([H, 1], f32)
            nc.vector.tensor_scalar(out=mean[:], in0=res[:, 0:1],
                                    scalar1=1.0 / N, scalar2=None,
                                    op0=mybir.AluOpType.mult)
            ex2 = sp.tile([H, 1], f32)
            nc.vector.tensor_scalar(out=ex2[:], in0=res[:, 1:2],
                                    scalar1=1.0 / N, scalar2=None,
                                    op0=mybir.AluOpType.mult)
            msq = sp.tile([H, 1], f32)
            nc.vector.tensor_tensor(out=msq[:], in0=mean[:], in1=mean[:],
                                    op=mybir.AluOpType.mult)
            var = sp.tile([H, 1], f32)
            nc.vector.tensor_tensor(out=var[:], in0=ex2[:], in1=msq[:],
                                    op=mybir.AluOpType.subtract)
            vare = sp.tile([H, 1], f32)
            nc.vector.tensor_scalar(out=vare[:], in0=var[:], scalar1=EPS,
                                    scalar2=None, op0=mybir.AluOpType.add)
            std = sp.tile([H, 1], f32)
            nc.scalar.activation(out=std[:], in_=vare[:],
                                 func=mybir.ActivationFunctionType.Sqrt)
            inv = sp.tile([H, 1], f32)
            nc.vector.reciprocal(inv[:], std[:])
            scale = sp.tile([H, 1], f32)
            nc.vector.tensor_tensor(out=scale[:], in0=gam_t[:], in1=inv[:],
                                    op=mybir.AluOpType.mult)
            mscale = sp.tile([H, 1], f32)
            nc.vector.tensor_tensor(out=mscale[:], in0=mean[:], in1=scale[:],
                                    op=mybir.AluOpType.mult)
            shift = sp.tile([H, 1], f32)
            nc.vector.tensor_tensor(out=shift[:], in0=bet_t[:], in1=mscale[:],
                                    op=mybir.AluOpType.subtract)
            w2 = sp.tile([H, OUT], f32)
            nc.vector.tensor_scalar(out=w2[:], in0=wlin_t[:], scalar1=scale[:],
                                    scalar2=None, op0=mybir.AluOpType.mult)
            w2h = sp.tile([H, OUT], f16)
            nc.vector.tensor_copy(out=w2h[:], in_=w2[:])
            psc = stp.tile([1, OUT], f32, tag="st", name="psc")
            nc.tensor.matmul(psc[:], shift[:], wlin_t[:], start=True, stop=True)
            cvec = sp.tile([1, OUT], f32)
            nc.vector.tensor_tensor(out=cvec[:], in0=psc[:], in1=blin_t[:],
                                    op=mybir.AluOpType.add)
            ones_t = sp.tile([1, 128], f32)
            nc.vector.memset(ones_t[:], 1.0)
            pscb = stp.tile([128, OUT], f32, tag="st", name="pscb")
            nc.tensor.matmul(pscb[:], ones_t[:], cvec[:], start=True, stop=True)
            cb2 = sp.tile([128, OUT], f32)
            nc.vector.tensor_copy(out=cb2[:], in_=pscb[:])

            # ---------------- logits + batched softmax tail ----------------
            lg = lgp.tile([128, NPAIR * 2], f32, tag="lg")
            for pg in range(NPAIR):
                nc.tensor.matmul(lg[:, pg * 2:(pg + 1) * 2],
                                 agg[:, pg * PC:(pg + 1) * PC], w2h[:],
                                 start=True, stop=True)
            nc.scalar.copy(out=Lt[:], in_=lg[:])

            lc2 = aggp.tile([128, NPAIR * 2], f32)
            nc.vector.tensor_tensor(
                out=lc2[:].rearrange("p (k o) -> p k o", o=2),
                in0=Lt[:].rearrange("p (k o) -> p k o", o=2),
                in1=bcast_mid(cb2[:], NPAIR),
                op=mybir.AluOpType.add)
            ee = aggp.tile([128, NPAIR * 2], f32)
            nc.scalar.activation(out=ee[:], in_=lc2[:],
                                 func=mybir.ActivationFunctionType.Exp)
            # exp(relu(x)) = max(exp(x), 1)
            nc.vector.tensor_scalar(out=ee[:], in0=ee[:], scalar1=1.0,
                                    scalar2=None, op0=mybir.AluOpType.max)
            ssm = aggp.tile([128, NPAIR], f32)
            nc.vector.tensor_reduce(
                out=ssm[:], in_=ee[:].rearrange("p (k o) -> p k o", o=2),
                axis=mybir.AxisListType.X, op=mybir.AluOpType.add)
            rin = aggp.tile([128, NPAIR], f32)
            nc.vector.reciprocal(rin[:], ssm[:])
            rm = aggp.tile([128, NPAIR * 2], f32)
            nc.vector.tensor_tensor(
                out=rm[:].rearrange("p (k o) -> p k o", o=2),
                in0=bcast_last(rin[:], 2),
                in1=mask_t[:].rearrange("p (k o) -> p k o", o=2),
                op=mybir.AluOpType.mult)
            pf = aggp.tile([128, NPAIR * 2], f32)
            nc.vector.tensor_tensor(out=pf[:], in0=ee[:], in1=rm[:],
                                    op=mybir.AluOpType.mult)
            nc.sync.dma_start(out=outd[:], in_=pf[:])

    nc.finalize()
    return nc


def build_in_maps(inputs, meta):
    state = np.asarray(inputs["state"], dtype=np.float32)
    Wm = np.asarray(inputs["W"], dtype=np.float32)
    gamma = np.asarray(inputs["gamma"], dtype=np.float32)
    beta = np.asarray(inputs["beta"], dtype=np.float32)
    Wlin = np.asarray(inputs["Wlin"], dtype=np.float32)
    blin = np.asarray(inputs["blin"], dtype=np.float32)
    mask = np.asarray(inputs["mask"])

    x = state.reshape(N, F_IN)
    # xpad layout: node n -> row (n%4)*BROWS + n//4
    xpad = np.zeros((XROWS, 128), dtype=np.float16)
    rows = (np.arange(N) % NBLK) * BROWS + np.arange(N) // NBLK
    xpad[rows, :F_IN] = x.astype(np.float16)

    core_of, pair_of, row_of = (meta["core_of"], meta["pair_of"],
                                meta["row_of"])
    maskc = np.zeros((NCORES, 128, NPAIR * 2), dtype=np.float32)
    mf = mask.astype(np.float32)
    maskc[core_of, row_of, pair_of * 2] = mf
    maskc[core_of, row_of, pair_of * 2 + 1] = mf

    in_maps = []
    for c in range(NCORES):
        in_maps.append(dict(
            xpad=xpad,
            idx=meta["idx16"][c],
            oh=meta["oh"][c].reshape(128, -1),
            mask=maskc[c],
            wmat=Wm.astype(np.float16),
            gam=gamma.reshape(H, 1),
            bet=beta.reshape(H, 1),
            wlin=Wlin,
            blin=blin.reshape(1, OUT),
        ))
    return in_maps


def unpack_out(meta, outs):
    core_of, pair_of, row_of = (meta["core_of"], meta["pair_of"],
                                meta["row_of"])
    nds = np.arange(N)
    actor = np.zeros((N, OUT), dtype=np.float32)
    for c in range(NCORES):
        o = outs[c]                                 # [128, NPAIR*2]
        sel = core_of == c
        actor[nds[sel], 0] = o[row_of[sel], pair_of[sel] * 2]
        actor[nds[sel], 1] = o[row_of[sel], pair_of[sel] * 2 + 1]
    return actor


def kernel(**inputs):
    edge_index = np.asarray(inputs["edge_index"])
    meta = _prep(edge_index)

    key = tuple(meta["segcols"].reshape(-1).tolist())
    if key not in _cache:
        _cache[key] = _build(meta)
    nc = _cache[key]

    in_maps = build_in_maps(inputs, meta)
    global _last_in_maps
    _last_in_maps = in_maps
    import os
    from concourse.bass_utils import run_bass_kernel_spmd
    if os.environ.get("KERNEL_TRACE"):
        import tempfile
        tcores = None
        if os.environ.get("KERNEL_TRACE_CORES"):
            tcores = [int(c) for c in
                      os.environ["KERNEL_TRACE_CORES"].split(",")]
        r = run_bass_kernel_spmd(nc, in_maps, list(range(NCORES)), trace=True,
                                 trace_cores=tcores,
                                 tmpdir=tempfile.mkdtemp(prefix="ktrace_"))
        print(f"HW exec time: {r.exec_time_ns} ns")
    else:
        r = run_bass_kernel_spmd(nc, in_maps, list(range(NCORES)), trace=False)

    return unpack_out(meta, [r.results[c]["out"] for c in range(NCORES)])
